# revision 40
# baseline (speedup 1.0000x reference)
"""Trainium2 Bass kernel for nn_KnnConstraint (ball-query KNN constraint loss).

Math (faithful to the reference):
  For each batch b and query point i: take the first K=20 points j (in index
  order) with ||x_i - x_j||^2 <= r^2, drop the first one, keep up to 19.
  For each kept (i, j):
      cd = ||x_i - x_j||, nd = ||c_i - c_j||, w = exp(-0.1 * nd^2)
      term = sqrt((cd - nd)^2 * w + 1e-20) ~= |cd - nd| * exp(-0.05 * nd^2)
  loss = mean over all B*N*19 slots (invalid slots contribute sqrt(1e-20)).

Kernel strategy (J-truncation + query-partition layout):
  Ranks <= 20 come from early j: a pair (i, j) contributes only if fewer
  than 20 in-ball points precede j. The device sweeps j < J=256 for all
  queries. Queries whose in-ball count over j<J is >= 21 are fully
  covered on-device (all rank-2..20 pairs lie below J); the rest are
  recomputed exactly on the host in vectorized numpy (transcendentals
  only on the <=19 selected pairs per query).

  Layout: queries on PARTITIONS, j on the free axis. Per core: 2048
  queries = 16 i-tiles of 128, processed in 4 chunks of [128, 4*J]:
    PE : d2 via augmented fp32r matmul (1 cyc/row vs fp32's ~10)
    ACT: cd = Sqrt(d2 + eps) -> fp16 ; q = Square(s - 11)
    DVE: w01 = (cd <= thr); s = rank via ONE tensor_tensor_scan per
         chunk (bmask resets the running state at i-tile boundaries);
         band = (q <= 90.25); m = band * w01; em = m * e_plane;
         u = cd - nd_plane; z = u * em
    ACT: az = |z| with accum_out -> per-query sums (last chunk: DVE
         tensor_reduce abs, shorter kernel tail)
  The chunk tail (z, counts, az) is emitted one chunk late so the
  in-order DVE queue never stalls on a cross-engine dependency.
  EPS_D2 = 4e-3 keeps Sqrt's argument positive under fp32r matmul
  cancellation noise (~1e-3) on the d2(i,i) = 0 self-pairs.
  Canonical nd / exp planes are batch-independent: host precomputes
  them once (cached) and streams them in as fp16 in tile layout, nd|e
  interleaved per chunk so each chunk is a single DMA.
  Host: covered-query sums from device accum + exact numpy fallback for
  uncovered queries + epsilon terms for invalid slots.
"""

import hashlib
import math

import numpy as np

N = 4096
B = 4
HALF = 2048
K = 20
P = 128
J = 256  # device j-truncation
NT = HALF // P  # 16 i-tiles per core
CHUNK = 4  # i-tiles per chunk
NCH = NT // CHUNK  # 4 chunks
NCORES = 8
SLOTS = K - 1  # 19
EPS_D2 = 4.0e-3  # must exceed fp32r matmul cancellation noise (~1e-3)

_CACHE = {}
_PLANES = {}


def _build_program(r2: float):
    import concourse.bass as bass  # noqa: F401
    import concourse.mybir as mybir
    from concourse import bacc
    from concourse.tile import TileContext

    f32 = mybir.dt.float32
    f32r = mybir.dt.float32r
    fp16 = mybir.dt.float16
    ALU = mybir.AluOpType
    ACT = mybir.ActivationFunctionType
    AX = mybir.AxisListType

    nc = bacc.Bacc(None, target_bir_lowering=False)
    # one input param: cols [0:HALF] queries aug [qx,qy,qz,sqq,1],
    # cols [HALF:HALF+J] points aug [-2px,-2py,-2pz,1,sqp]
    allin = nc.declare_dram_parameter("allin", [5, HALF + J], f32r, isOutput=False)
    # planes interleaved per chunk: [p, c, 0, :] = nd chunk c, [p, c, 1, :] = e
    plcat = nc.declare_dram_parameter("plcat", [P, NT * J * 2], fp16, isOutput=False)
    # outputs merged: cols [0:NT] acc sums, [NT:2*NT] counts (both f32)
    out_all = nc.declare_dram_parameter("out_all", [P, 2 * NT], f32, isOutput=True)

    cd_thr = float(math.sqrt(r2 + EPS_D2))
    CW = CHUNK * J  # chunk row width

    with TileContext(nc) as tc:
        with (
            tc.tile_pool(name="const", bufs=1) as cpool,
            tc.tile_pool(name="planes", bufs=3) as plpool,
            tc.tile_pool(name="work", bufs=2) as wpool,
            tc.tile_pool(name="pd", bufs=2, space="PSUM") as pdpool,
        ):
            allin_sb = cpool.tile_from(allin[:, :])
            qaug_sb = allin_sb[:, 0:HALF]
            paug_sb = allin_sb[:, HALF : HALF + J]
            eps_bias = cpool.tile([P, 1], f32)
            nc.vector.memset(eps_bias, EPS_D2)
            neg11 = cpool.tile([P, 1], f32)
            nc.vector.memset(neg11, -11.0)
            outS = cpool.tile([P, 2 * NT], f32)
            accS = outS[:, 0:NT]
            cntS = outS[:, NT : 2 * NT]
            # scan reset mask: 0 at i-tile boundary columns, 1 elsewhere
            bmask = cpool.tile([P, CW], fp16)
            nc.vector.memset(bmask, 1.0)
            for k in range(1, CHUNK):
                nc.vector.memset(bmask[:, k * J : k * J + 1], 0.0)

            def emit_tail(st):
                # deferred tail of a chunk: z needs gpsimd's em -- emitting
                # it one chunk late keeps the in-order DVE queue from
                # stalling on the cross-engine dependency
                c, s, u, em = st
                z = wpool.tile([P, CW], fp16, tag="z")
                nc.vector.tensor_tensor(z, u, em, ALU.mult)
                # per-query in-ball count over j<J = last scan value of
                # each i-tile (strided slice, 4 values in one instr)
                nc.vector.tensor_scalar(
                    cntS[:, CHUNK * c : CHUNK * (c + 1)],
                    s[:, J - 1 :: J],
                    0.0,
                    None,
                    ALU.add,
                )
                if c == NCH - 1:
                    # last chunk: reduce on DVE -- shorter kernel tail than
                    # the serial ACT accumulate chain
                    for k in range(CHUNK):
                        t = CHUNK * c + k
                        sl = slice(k * J, (k + 1) * J)
                        nc.vector.tensor_reduce(
                            accS[:, t : t + 1], z[:, sl], AX.X, ALU.add,
                            apply_absolute_value=True,
                        )
                    return
                az = wpool.tile([P, CW], fp16, tag="az")
                for k in range(CHUNK):
                    t = CHUNK * c + k
                    sl = slice(k * J, (k + 1) * J)
                    nc.scalar.activation(
                        az[:, sl], z[:, sl], ACT.Abs, bias=0.0, scale=1.0,
                        accum_out=accS[:, t : t + 1],
                    )

            pending = None
            for c in range(NCH):
                pl_c = plpool.tile([P, 2 * CW], fp16, tag="pl")
                nc.sync.dma_start(pl_c, plcat[:, c * 2 * CW : (c + 1) * 2 * CW])
                nd_c = pl_c[:, 0:CW]
                e_c = pl_c[:, CW : 2 * CW]

                psum_d = pdpool.tile([P, CW], f32, tag="pd")
                for k in range(CHUNK):
                    t = CHUNK * c + k
                    nc.tensor.matmul(
                        psum_d[:, k * J : (k + 1) * J],
                        qaug_sb[:, t * P : (t + 1) * P],
                        paug_sb[:, :],
                        start=True,
                        stop=True,
                    )
                cd = wpool.tile([P, CW], fp16, tag="cd")
                nc.scalar.activation(
                    cd, psum_d, ACT.Sqrt, bias=eps_bias[:, :], scale=1.0
                )
                w01 = wpool.tile([P, CW], fp16, tag="w01")
                nc.vector.tensor_scalar(w01, cd, cd_thr, None, ALU.is_le)
                # rank: one cumsum over the whole chunk; bmask resets the
                # running state at each i-tile boundary
                s = wpool.tile([P, CW], fp16, tag="s")
                nc.vector.tensor_tensor_scan(
                    s, bmask, w01, 0.0, ALU.mult, ALU.add
                )
                # band 2<=s<=20  <=>  (s-11)^2 <= 90.25 (scalar engine;
                # fp16 overflow at s>255 gives inf -> band 0, still correct)
                q = wpool.tile([P, CW], fp16, tag="q")
                nc.scalar.activation(q, s, ACT.Square, bias=neg11[:, :], scale=1.0)
                band = wpool.tile([P, CW], fp16, tag="band")
                nc.vector.tensor_scalar(band, q, 90.25, None, ALU.is_le)
                m = wpool.tile([P, CW], fp16, tag="m")
                nc.vector.tensor_tensor(m, band, w01, ALU.mult)
                u = wpool.tile([P, CW], fp16, tag="u")
                nc.vector.tensor_tensor(u, cd, nd_c, ALU.subtract)
                # em on DVE for all chunks: keeping gpsimd entirely idle
                # lets the end-of-kernel drain barrier finish sooner
                em = wpool.tile([P, CW], fp16, tag="emd")
                nc.vector.tensor_tensor(em, m, e_c, ALU.mult)
                if pending is not None:
                    emit_tail(pending)
                pending = (c, s, u, em)
            emit_tail(pending)

            nc.default_dma_engine.dma_start(out_all[:, :], outS[:, :])
    nc.compile()
    return nc


def _get_planes(canno):
    """Per-half interleaved plane tensor [128, NT*J*2] fp16 (per-chunk
    nd then e blocks), cached by canonical-cloud hash."""
    key = hashlib.sha1(canno.tobytes()).hexdigest()
    if key in _PLANES:
        return _PLANES[key]
    c = canno.astype(np.float32)
    csq = (c * c).sum(-1)
    nd2 = csq[:, None] + csq[None, :J] - 2.0 * (c @ c[:J].T)  # [N, J]
    np.maximum(nd2, 0.0, out=nd2)
    nd = np.sqrt(nd2)
    e = np.exp(-0.05 * nd2)
    CW = CHUNK * J
    out = {}
    for h in range(2):
        rows = slice(h * HALF, (h + 1) * HALF)
        # tile layout [P, NT*J] then group per chunk and interleave nd|e
        ndt = (nd[rows].astype(np.float16)
               .reshape(NT, P, J).transpose(1, 0, 2).reshape(P, NT * J))
        et = (e[rows].astype(np.float16)
              .reshape(NT, P, J).transpose(1, 0, 2).reshape(P, NT * J))
        cat = np.concatenate(
            [ndt.reshape(P, NCH, 1, CW), et.reshape(P, NCH, 1, CW)], axis=2
        ).reshape(P, NT * J * 2)
        out[h] = np.ascontiguousarray(cat)
    _PLANES.clear()
    _PLANES[key] = out
    return out


def _prep_core_inputs(xyz, core, planes):
    b, h = core // 2, core % 2
    pts = xyz[b]  # [N, 3]
    sq = (pts * pts).sum(-1)
    q = pts[h * HALF : (h + 1) * HALF]
    sqq = sq[h * HALF : (h + 1) * HALF]
    qaug = np.stack(
        [q[:, 0], q[:, 1], q[:, 2], sqq, np.ones(HALF, np.float32)]
    )
    pj = pts[:J]
    paug = np.stack(
        [-2.0 * pj[:, 0], -2.0 * pj[:, 1], -2.0 * pj[:, 2],
         np.ones(J, np.float32), sq[:J]]
    )
    allin = np.concatenate([qaug, paug], axis=1).astype(np.float32)
    return {
        "allin": np.ascontiguousarray(allin),
        "plcat": planes[h],
    }


def _host_fallback(xyz, canno, r2, fb_mask):
    """Exact recompute for fallback queries (vectorized numpy).
    Transcendentals only on selected pairs (<=19 per query).
    Returns (term_sum, n_valid) over fallback queries."""
    canno = canno.astype(np.float32)
    csq = (canno * canno).sum(-1)
    tot = 0.0
    nval = 0
    for b in range(B):
        idx = np.nonzero(fb_mask[b])[0]
        if idx.size == 0:
            continue
        pts = xyz[b]
        sq = (pts * pts).sum(-1)
        d2 = sq[idx, None] + sq[None, :] - 2.0 * (pts[idx] @ pts.T)
        within = d2 <= r2
        s = np.cumsum(within, axis=1, dtype=np.int32)
        sel = within & (s >= 2) & (s <= K)
        cnt = s[:, -1]
        nval += int(np.minimum(np.maximum(cnt - 1, 0), SLOTS).sum())
        ri, ci = np.nonzero(sel)  # <= 19 per row
        gq = idx[ri]
        cd = np.sqrt(np.maximum(d2[ri, ci], 0.0))
        nd2 = (csq[gq] + csq[ci]
               - 2.0 * (canno[gq] * canno[ci]).sum(-1))
        np.maximum(nd2, 0.0, out=nd2)
        nd = np.sqrt(nd2)
        tot += float((np.abs(cd - nd) * np.exp(-0.05 * nd2)).sum())
    return tot, nval


def kernel(xyz, canno_xyz, radius, _trace=False, _return_res=False):
    from concourse.bass_utils import run_bass_kernel_spmd

    xyz = np.asarray(xyz, np.float32)
    canno = np.asarray(canno_xyz, np.float32)
    r2 = float(np.asarray(radius, np.float32)) ** 2

    key = ("v8", r2)
    if key not in _CACHE:
        _CACHE[key] = _build_program(r2)
    nc = _CACHE[key]
    planes = _get_planes(canno)
    in_maps = [_prep_core_inputs(xyz, c, planes) for c in range(NCORES)]
    res = run_bass_kernel_spmd(nc, in_maps, list(range(NCORES)), trace=_trace)

    # assemble: device sums for covered queries, exact fallback for the rest
    dev_sum = 0.0
    covered_total = 0
    fb_mask = np.zeros((B, N), bool)
    for c in range(NCORES):
        b, h = c // 2, c % 2
        out = np.asarray(res.results[c]["out_all"], np.float64)  # [128, 32]
        acc = out[:, :NT]
        cnt = out[:, NT:]
        cov = cnt >= 20.5  # count_J >= 21
        dev_sum += float(acc[cov].sum())
        covered_total += int(cov.sum())
        # query id = h*HALF + t*128 + p  (cov is [p, t])
        fb = ~cov  # [128, 16]
        pidx, tidx = np.nonzero(fb)
        fb_mask[b, h * HALF + tidx * P + pidx] = True

    fb_sum, fb_nval = _host_fallback(xyz, canno, r2, fb_mask)
    n_valid = covered_total * SLOTS + fb_nval
    total_slots = B * N * SLOTS
    eps_term = float(np.sqrt(np.float64(np.float32(1e-20))))
    loss = (dev_sum + fb_sum + (total_slots - n_valid) * eps_term) / total_slots
    out = np.array(loss, dtype=np.float32)
    if _return_res:
        return out, res
    return out


# revision 42
# speedup vs baseline: 1.1664x; 1.1664x over previous
"""Trainium2 Bass kernel for nn_KnnConstraint (ball-query KNN constraint loss).

Math (faithful to the reference):
  For each batch b and query point i: take the first K=20 points j (in index
  order) with ||x_i - x_j||^2 <= r^2, drop the first one, keep up to 19.
  For each kept (i, j):
      cd = ||x_i - x_j||, nd = ||c_i - c_j||, w = exp(-0.1 * nd^2)
      term = sqrt((cd - nd)^2 * w + 1e-20) ~= |cd - nd| * exp(-0.05 * nd^2)
  loss = mean over all B*N*19 slots (invalid slots contribute sqrt(1e-20)).

Kernel strategy (J-truncation + query-partition layout):
  Ranks <= 20 come from early j: a pair (i, j) contributes only if fewer
  than 20 in-ball points precede j. The device sweeps j < J=256 for all
  queries. Queries whose in-ball count over j<J is >= 21 are fully
  covered on-device (all rank-2..20 pairs lie below J); the rest are
  recomputed exactly on the host in vectorized numpy (transcendentals
  only on the <=19 selected pairs per query).

  Layout: queries on PARTITIONS, j on the free axis. Per core: 2048
  queries = 16 i-tiles of 128, processed in 4 chunks of [128, 4*J]:
    PE : d2 via augmented fp32r matmul (1 cyc/row vs fp32's ~10)
    ACT: cd = Sqrt(d2 + eps) -> fp16 ; q = Square(s - 11)
    DVE: w01 = (cd <= thr); s = rank via ONE tensor_tensor_scan per
         chunk (bmask resets the running state at i-tile boundaries);
         band = (q <= 90.25); m = band * w01; em = m * e_plane;
         u = cd - nd_plane; z = u * em
    ACT: az = |z| with accum_out -> per-query sums (last chunk: DVE
         tensor_reduce abs, shorter kernel tail)
  The chunk tail (z, counts, az) is emitted one chunk late so the
  in-order DVE queue never stalls on a cross-engine dependency.
  EPS_D2 = 4e-3 keeps Sqrt's argument positive under fp32r matmul
  cancellation noise (~1e-3) on the d2(i,i) = 0 self-pairs.
  Canonical nd / exp planes are batch-independent: host precomputes
  them once (cached) and streams them in as fp16 in tile layout, nd|e
  interleaved per chunk so each chunk is a single DMA.
  Host: covered-query sums from device accum + exact numpy fallback for
  uncovered queries + epsilon terms for invalid slots.
"""

import hashlib
import math

import numpy as np

N = 4096
B = 4
HALF = 2048
K = 20
P = 128
J = 256  # device j-truncation
NT = HALF // P  # 16 i-tiles per core
CHUNK = 4  # i-tiles per chunk
NCH = NT // CHUNK  # 4 chunks
NCORES = 8
SLOTS = K - 1  # 19
EPS_D2 = 4.0e-3  # must exceed fp32r matmul cancellation noise (~1e-3)

_CACHE = {}
_PLANES = {}


def _build_program(r2: float):
    import concourse.bass as bass  # noqa: F401
    import concourse.mybir as mybir
    from concourse import bacc
    from concourse.tile import TileContext

    f32 = mybir.dt.float32
    f32r = mybir.dt.float32r
    fp16 = mybir.dt.float16
    ALU = mybir.AluOpType
    ACT = mybir.ActivationFunctionType
    AX = mybir.AxisListType

    nc = bacc.Bacc(None, target_bir_lowering=False)
    # one input param: cols [0:HALF] queries aug [qx,qy,qz,sqq,1],
    # cols [HALF:HALF+J] points aug [-2px,-2py,-2pz,1,sqp]
    allin = nc.declare_dram_parameter("allin", [5, HALF + J], f32r, isOutput=False)
    # planes interleaved per chunk: [p, c, 0, :] = nd chunk c, [p, c, 1, :] = e
    plcat = nc.declare_dram_parameter("plcat", [P, NT * J * 2], fp16, isOutput=False)
    # outputs merged: cols [0:NT] acc sums, [NT:2*NT] counts (both f32)
    out_all = nc.declare_dram_parameter("out_all", [P, 2 * NT], f32, isOutput=True)

    cd_thr = float(math.sqrt(r2 + EPS_D2))
    CW = CHUNK * J  # chunk row width

    with TileContext(nc) as tc:
        with (
            tc.tile_pool(name="const", bufs=1) as cpool,
            tc.tile_pool(name="planes", bufs=2) as plpool,
            tc.tile_pool(name="work", bufs=2) as wpool,
            tc.tile_pool(name="pd", bufs=2, space="PSUM") as pdpool,
        ):
            allin_sb = cpool.tile_from(allin[:, :])
            qaug_sb = allin_sb[:, 0:HALF]
            paug_sb = allin_sb[:, HALF : HALF + J]
            eps_bias = cpool.tile([P, 1], f32)
            nc.vector.memset(eps_bias, EPS_D2)
            neg11 = cpool.tile([P, 1], f32)
            nc.vector.memset(neg11, -11.0)
            outS = cpool.tile([P, 2 * NT], f32)
            accS = outS[:, 0:NT]
            cntS = outS[:, NT : 2 * NT]
            # scan reset mask: 0 at i-tile boundary columns, 1 elsewhere
            bmask = cpool.tile([P, CW], fp16)
            nc.vector.memset(bmask, 1.0)
            for k in range(1, CHUNK):
                nc.vector.memset(bmask[:, k * J : k * J + 1], 0.0)

            def emit_tail(st):
                # deferred tail of a chunk: z needs gpsimd's em -- emitting
                # it one chunk late keeps the in-order DVE queue from
                # stalling on the cross-engine dependency
                c, s, u, em = st
                z = wpool.tile([P, CW], fp16, tag="z")
                nc.vector.tensor_tensor(z, u, em, ALU.mult)
                # per-query in-ball count over j<J = last scan value of
                # each i-tile (strided slice, 4 values in one instr)
                nc.vector.tensor_scalar(
                    cntS[:, CHUNK * c : CHUNK * (c + 1)],
                    s[:, J - 1 :: J],
                    0.0,
                    None,
                    ALU.add,
                )
                if c == NCH - 1:
                    # last chunk: reduce on DVE -- shorter kernel tail than
                    # the serial ACT accumulate chain
                    for k in range(CHUNK):
                        t = CHUNK * c + k
                        sl = slice(k * J, (k + 1) * J)
                        nc.vector.tensor_reduce(
                            accS[:, t : t + 1], z[:, sl], AX.X, ALU.add,
                            apply_absolute_value=True,
                        )
                    return
                az = wpool.tile([P, CW], fp16, tag="az")
                for k in range(CHUNK):
                    t = CHUNK * c + k
                    sl = slice(k * J, (k + 1) * J)
                    nc.scalar.activation(
                        az[:, sl], z[:, sl], ACT.Abs, bias=0.0, scale=1.0,
                        accum_out=accS[:, t : t + 1],
                    )

            pending = None
            for c in range(NCH):
                pl_c = plpool.tile([P, 2 * CW], fp16, tag="pl")
                nc.sync.dma_start(pl_c, plcat[:, c * 2 * CW : (c + 1) * 2 * CW])
                nd_c = pl_c[:, 0:CW]
                e_c = pl_c[:, CW : 2 * CW]

                psum_d = pdpool.tile([P, CW], f32, tag="pd")
                for k in range(CHUNK):
                    t = CHUNK * c + k
                    nc.tensor.matmul(
                        psum_d[:, k * J : (k + 1) * J],
                        qaug_sb[:, t * P : (t + 1) * P],
                        paug_sb[:, :],
                        start=True,
                        stop=True,
                    )
                cd = wpool.tile([P, CW], fp16, tag="cd")
                nc.scalar.activation(
                    cd, psum_d, ACT.Sqrt, bias=eps_bias[:, :], scale=1.0
                )
                w01 = wpool.tile([P, CW], fp16, tag="w01")
                nc.vector.tensor_scalar(w01, cd, cd_thr, None, ALU.is_le)
                # rank: one cumsum over the whole chunk; bmask resets the
                # running state at each i-tile boundary
                s = wpool.tile([P, CW], fp16, tag="s")
                nc.vector.tensor_tensor_scan(
                    s, bmask, w01, 0.0, ALU.mult, ALU.add
                )
                # band 2<=s<=20  <=>  (s-11)^2 <= 90.25 (scalar engine;
                # fp16 overflow at s>255 gives inf -> band 0, still correct)
                q = wpool.tile([P, CW], fp16, tag="q")
                nc.scalar.activation(q, s, ACT.Square, bias=neg11[:, :], scale=1.0)
                band = wpool.tile([P, CW], fp16, tag="band")
                nc.vector.tensor_scalar(band, q, 90.25, None, ALU.is_le)
                m = wpool.tile([P, CW], fp16, tag="m")
                nc.vector.tensor_tensor(m, band, w01, ALU.mult)
                u = wpool.tile([P, CW], fp16, tag="u")
                nc.vector.tensor_tensor(u, cd, nd_c, ALU.subtract)
                # em on DVE for all chunks: keeping gpsimd entirely idle
                # lets the end-of-kernel drain barrier finish sooner
                em = wpool.tile([P, CW], fp16, tag="emd")
                nc.vector.tensor_tensor(em, m, e_c, ALU.mult)
                if pending is not None:
                    emit_tail(pending)
                pending = (c, s, u, em)
            emit_tail(pending)

            nc.sync.dma_start(out_all[:, :], outS[:, :])
    nc.compile()
    return nc


def _get_planes(canno):
    """Per-half interleaved plane tensor [128, NT*J*2] fp16 (per-chunk
    nd then e blocks), cached by canonical-cloud hash."""
    key = hashlib.sha1(canno.tobytes()).hexdigest()
    if key in _PLANES:
        return _PLANES[key]
    c = canno.astype(np.float32)
    csq = (c * c).sum(-1)
    nd2 = csq[:, None] + csq[None, :J] - 2.0 * (c @ c[:J].T)  # [N, J]
    np.maximum(nd2, 0.0, out=nd2)
    nd = np.sqrt(nd2)
    e = np.exp(-0.05 * nd2)
    CW = CHUNK * J
    out = {}
    for h in range(2):
        rows = slice(h * HALF, (h + 1) * HALF)
        # tile layout [P, NT*J] then group per chunk and interleave nd|e
        ndt = (nd[rows].astype(np.float16)
               .reshape(NT, P, J).transpose(1, 0, 2).reshape(P, NT * J))
        et = (e[rows].astype(np.float16)
              .reshape(NT, P, J).transpose(1, 0, 2).reshape(P, NT * J))
        cat = np.concatenate(
            [ndt.reshape(P, NCH, 1, CW), et.reshape(P, NCH, 1, CW)], axis=2
        ).reshape(P, NT * J * 2)
        out[h] = np.ascontiguousarray(cat)
    _PLANES.clear()
    _PLANES[key] = out
    return out


def _prep_core_inputs(xyz, core, planes):
    b, h = core // 2, core % 2
    pts = xyz[b]  # [N, 3]
    sq = (pts * pts).sum(-1)
    q = pts[h * HALF : (h + 1) * HALF]
    sqq = sq[h * HALF : (h + 1) * HALF]
    qaug = np.stack(
        [q[:, 0], q[:, 1], q[:, 2], sqq, np.ones(HALF, np.float32)]
    )
    pj = pts[:J]
    paug = np.stack(
        [-2.0 * pj[:, 0], -2.0 * pj[:, 1], -2.0 * pj[:, 2],
         np.ones(J, np.float32), sq[:J]]
    )
    allin = np.concatenate([qaug, paug], axis=1).astype(np.float32)
    return {
        "allin": np.ascontiguousarray(allin),
        "plcat": planes[h],
    }


def _host_fallback(xyz, canno, r2, fb_mask):
    """Exact recompute for fallback queries (vectorized numpy).
    Transcendentals only on selected pairs (<=19 per query).
    Returns (term_sum, n_valid) over fallback queries."""
    canno = canno.astype(np.float32)
    csq = (canno * canno).sum(-1)
    tot = 0.0
    nval = 0
    for b in range(B):
        idx = np.nonzero(fb_mask[b])[0]
        if idx.size == 0:
            continue
        pts = xyz[b]
        sq = (pts * pts).sum(-1)
        d2 = sq[idx, None] + sq[None, :] - 2.0 * (pts[idx] @ pts.T)
        within = d2 <= r2
        s = np.cumsum(within, axis=1, dtype=np.int32)
        sel = within & (s >= 2) & (s <= K)
        cnt = s[:, -1]
        nval += int(np.minimum(np.maximum(cnt - 1, 0), SLOTS).sum())
        ri, ci = np.nonzero(sel)  # <= 19 per row
        gq = idx[ri]
        cd = np.sqrt(np.maximum(d2[ri, ci], 0.0))
        nd2 = (csq[gq] + csq[ci]
               - 2.0 * (canno[gq] * canno[ci]).sum(-1))
        np.maximum(nd2, 0.0, out=nd2)
        nd = np.sqrt(nd2)
        tot += float((np.abs(cd - nd) * np.exp(-0.05 * nd2)).sum())
    return tot, nval


def kernel(xyz, canno_xyz, radius, _trace=False, _return_res=False):
    from concourse.bass_utils import run_bass_kernel_spmd

    xyz = np.asarray(xyz, np.float32)
    canno = np.asarray(canno_xyz, np.float32)
    r2 = float(np.asarray(radius, np.float32)) ** 2

    key = ("v8", r2)
    if key not in _CACHE:
        _CACHE[key] = _build_program(r2)
    nc = _CACHE[key]
    planes = _get_planes(canno)
    in_maps = [_prep_core_inputs(xyz, c, planes) for c in range(NCORES)]
    res = run_bass_kernel_spmd(nc, in_maps, list(range(NCORES)), trace=_trace)

    # assemble: device sums for covered queries, exact fallback for the rest
    dev_sum = 0.0
    covered_total = 0
    fb_mask = np.zeros((B, N), bool)
    for c in range(NCORES):
        b, h = c // 2, c % 2
        out = np.asarray(res.results[c]["out_all"], np.float64)  # [128, 32]
        acc = out[:, :NT]
        cnt = out[:, NT:]
        cov = cnt >= 20.5  # count_J >= 21
        dev_sum += float(acc[cov].sum())
        covered_total += int(cov.sum())
        # query id = h*HALF + t*128 + p  (cov is [p, t])
        fb = ~cov  # [128, 16]
        pidx, tidx = np.nonzero(fb)
        fb_mask[b, h * HALF + tidx * P + pidx] = True

    fb_sum, fb_nval = _host_fallback(xyz, canno, r2, fb_mask)
    n_valid = covered_total * SLOTS + fb_nval
    total_slots = B * N * SLOTS
    eps_term = float(np.sqrt(np.float64(np.float32(1e-20))))
    loss = (dev_sum + fb_sum + (total_slots - n_valid) * eps_term) / total_slots
    out = np.array(loss, dtype=np.float32)
    if _return_res:
        return out, res
    return out
